# revision 33
# baseline (speedup 1.0000x reference)
"""LoRA MultiheadAttention on 8 Trainium2 NeuronCores (Bass/Tile) — v3.

Sharding: core c = (batch n = c//2, head-group hg = c%2); each core handles
6 of 12 heads for one of 4 batches. LoRA is folded into the projection
weights on the host (exact identity). f16 activations/weights from host;
f16 output partials summed on host.

v3 structure (vs v2 @ 387µs): every TensorE stage is arranged so matmuls
alternate PE row groups, which (a) lets LDWEIGHTS of one matmul overlap
the previous matmul's streaming (they serialize when row groups collide:
~107ns tax per matmul in v2) and (b) lets pairs of K=64 matmuls stream
CONCURRENTLY through complementary halves of the PE array:
- Scores: the head pair (2p, 2p+1) lives at qkT partitions 0-63/64-127,
  so interleaving the two heads' score matmuls runs them on row groups
  0-1/2-3 concurrently. Processed in l-quarter groups of st-pairs so the
  attention tiles for one l-quarter can be consumed (attnV) while the
  next fills — bounds SBUF to ~2 groups.
- Projections / attnV / out-proj contract over K=128: split into K=64
  halves accumulated into SEPARATE psum half-tiles (cross-row-group
  accumulation into one psum region is a device error — probed), then
  combined by the DVE op that was already there (bias-add / output copy).
- Softmax denominators ride as a 65th v row; reciprocal via
  reciprocal_approx_fast on a pair-packed [64, L] tile; broadcast to the
  128 output rows via one K=64 matmul against a constant selection
  matrix; all partition starts 32-aligned.
- attnV chains and normalization are generator-interleaved into the
  next group's scores/exp emission so TensorE works through the
  ScalarE-bound exp stretches.
"""
import numpy as np

import concourse.bass as bass
import concourse.tile as tile
from concourse import bacc, mybir
from concourse.bass_utils import run_bass_kernel_spmd

L, N, E, H, R = 2048, 4, 768, 12, 16
ALPHA = 16.0
LORA_SCALE = ALPHA / R
HD = E // H          # 64
HG = 2
HPG = H // HG        # 6 heads per group
EG = E // HG         # 384
NC_ = 8
F32 = mybir.dt.float32
F16 = mybir.dt.float16
SCALE = 1.0 / float(np.sqrt(HD))
ADD = mybir.AluOpType.add

KC = E // 128        # 6 contraction chunks
EC = EG // 128       # 3 output chunks (= head pairs)
LT = L // 128        # 16 s tiles
VW = HPG * (HD + 1)  # 390

_CACHED = {}


def _build():
    nc = bacc.Bacc()
    xqT = nc.dram_tensor("xqT", [E, L], F16, kind="ExternalInput")
    xkT = nc.dram_tensor("xkT", [E, L], F16, kind="ExternalInput")
    xvT = nc.dram_tensor("xvT", [E, L], F16, kind="ExternalInput")
    wqT = nc.dram_tensor("wqT", [E, EG], F16, kind="ExternalInput")
    wkT = nc.dram_tensor("wkT", [E, EG], F16, kind="ExternalInput")
    wvT = nc.dram_tensor("wvT", [E, EG], F16, kind="ExternalInput")
    woT = nc.dram_tensor("woT", [EG, E], F16, kind="ExternalInput")
    bq = nc.dram_tensor("bq", [EG], F32, kind="ExternalInput")
    bk = nc.dram_tensor("bk", [EG], F32, kind="ExternalInput")
    out = nc.dram_tensor("out", [E, L], F16, kind="ExternalOutput")

    with tile.TileContext(nc) as tc:
        with (
            tc.tile_pool(name="big", bufs=44) as big,
            tc.tile_pool(name="persist", bufs=1) as persist,
            tc.tile_pool(name="small", bufs=1) as small,
            tc.tile_pool(name="outsb", bufs=4) as outsb_pool,
            tc.tile_pool(name="psum", bufs=1, space="PSUM") as psum,
        ):
            LO, HI = slice(0, 64), slice(64, 128)

            # ---- weights / constants (x DMAs are issued first, in
            # load_x below, so the first projection isn't queued behind
            # weight traffic it doesn't need yet) ----
            w16 = {}
            def load_w(pname, wdram):
                for j in range(KC):
                    wt = persist.tile([128, EG], F16, name=f"w16_{pname}{j}")
                    nc.sync.dma_start(wt[:], wdram[j * 128:(j + 1) * 128, :])
                    w16[pname, j] = wt
            wo16 = []
            bias_t = {}
            def load_bias(bname, bdram):
                for j in range(EC):
                    bt = persist.tile([128, 1], F32, name=f"b_{bname}{j}")
                    nc.sync.dma_start(bt[:], bdram[j * 128:(j + 1) * 128])
                    bias_t[bname, j] = bt
            esel = persist.tile([64, 128], F16, name="esel")
            nc.vector.memset(esel[:], 0.0)
            nc.vector.memset(esel[0:1, 0:64], 1.0)
            nc.vector.memset(esel[32:33, 64:128], 1.0)

            qkT = {}
            v_aug = [None] * LT
            oT = [persist.tile([128, L], F16, name=f"oT{j}")
                  for j in range(EC)]
            d2 = {p: small.tile([64, L], F32, name=f"d2_{p}")
                  for p in range(EC)}
            for p in range(EC):
                nc.vector.memset(d2[p][:], 1.0)
            attn = {}    # (head, stp, lq) -> [128, 1024] f16: st-pair halves

            # ---- input staging: [128, 1024] ring tiles, 2 per E-chunk ----
            x16 = {}
            def load_x(pname, xdram):
                for j in range(KC):
                    for lh in range(2):
                        xt = big.tile([128, 1024], F16, tag="big", name="x16")
                        nc.sync.dma_start(
                            xt[:], xdram[j * 128:(j + 1) * 128,
                                         lh * 1024:(lh + 1) * 1024])
                        x16[pname, j, lh] = xt

            def proj_qk(pname, e):
                dst = persist.tile([128, L], F16, name=f"{pname}T{e}")
                qkT[pname, e] = dst
                for lc in range(2):
                    mm = psum.tile([128, 1024], F32, tag="sc", bufs=2,
                                   name="mm_proj")
                    for half in range(2):
                        xsl = slice(half * 512, (half + 1) * 512)
                        for kk in range(KC):
                            nc.tensor.matmul(
                                mm[:, xsl],
                                w16[pname, kk][:, e * 128:(e + 1) * 128],
                                x16[pname, kk, lc][:, xsl],
                                start=(kk == 0), stop=(kk == KC - 1))
                    nc.vector.tensor_scalar_add(
                        dst[:, lc * 1024:(lc + 1) * 1024], mm[:],
                        bias_t[pname, e][:])
                    yield

            def proj_v(st):
                mm = psum.tile([128, 1024], F32, tag="sc", bufs=2,
                               name="mm_vproj")
                lh, xo = st // 8, (st % 8) * 128
                for kk in range(KC):
                    nc.tensor.matmul(
                        mm[:, 0:EG], x16["v", kk, lh][:, xo:xo + 128],
                        w16["v", kk][:],
                        start=(kk == 0), stop=(kk == KC - 1))
                vt = persist.tile([128, VW], F16, name=f"v_aug{st}")
                grp = vt.rearrange("p (h c) -> p h c", c=HD + 1)
                nc.vector.tensor_copy(
                    grp[:, :, 0:HD],
                    mm[:, 0:EG].rearrange("p (h c) -> p h c", c=HD))
                nc.vector.memset(grp[:, :, HD:HD + 1], 1.0)
                v_aug[st] = vt

            def scores_exp(p, lq, background):
                """Scores+exp for head pair p over l-quarter lq.

                Both heads' score matmuls write halves of ONE psum tile so
                they issue back-to-back with no inter-tile semaphore — the
                row-group-complementary pair (qkT partitions 0-63 / 64-127)
                then streams concurrently through the PE array.
                """
                qk = [(qkT["q", p][j * 64:(j + 1) * 64, :],
                       qkT["k", p][j * 64:(j + 1) * 64, :]) for j in range(2)]
                ls = slice(lq * 512, (lq + 1) * 512)
                for st in range(LT):
                    T = psum.tile([128, 1024], F32, tag="sc", bufs=2,
                                  name="mm_sc")
                    for j in range(2):
                        qs, ks = qk[j]
                        nc.tensor.matmul(
                            T[:, j * 512:(j + 1) * 512],
                            ks[:, st * 128:(st + 1) * 128],
                            qs[:, ls],
                            start=True, stop=True)
                    at = big.tile([128, 1024], F16, tag="big", name="attn")
                    attn[p, st, lq] = at
                    nc.scalar.activation(
                        at[:], T[:],
                        mybir.ActivationFunctionType.Exp, scale=SCALE)
                    pulls = 5 if (p, lq) == (0, 0) else 3
                    for _ in range(pulls):
                        next(background, None)

            def attnv(h, lq):
                """attnV chains for head h, l-quarter lq (split-K halves)."""
                et, j = h // 2, h % 2
                vsl = slice(h * (HD + 1), (h + 1) * (HD + 1))
                ch = psum.tile([128, 1024], F32, tag="av", bufs=2, name="ch")
                for st in range(LT):
                    at = attn[et, st, lq]
                    nc.tensor.matmul(
                        ch[0:65, 0:512], v_aug[st][:, vsl],
                        at[:, j * 512:(j + 1) * 512],
                        start=(st == 0), stop=(st == LT - 1))
                    yield
                ls = slice(lq * 512, (lq + 1) * 512)
                nc.vector.tensor_copy(
                    oT[et][j * 64:(j + 1) * 64, ls], ch[0:64, 0:512])
                nc.vector.tensor_copy(
                    d2[et][32 * j:32 * j + 1, ls], ch[64:65, 0:512])
                yield

            def norm(p):
                rec32 = small.tile([64, L], F32, tag="rec32", bufs=1,
                                   name="rec32")
                rec2 = small.tile([64, L], F16, tag="rec2", bufs=1,
                                  name="rec2")
                nc.vector.reciprocal_approx_fast(rec32[:], d2[p][:])
                nc.vector.tensor_copy(rec2[:], rec32[:])
                yield
                for lq in range(4):
                    bc = psum.tile([128, 1024], F32, tag="av", bufs=2,
                                   name="bc")
                    ls = slice(lq * 512, (lq + 1) * 512)
                    nc.tensor.matmul(bc[:, 0:512], esel[:], rec2[:, ls],
                                     start=True, stop=True)
                    nc.vector.tensor_mul(oT[p][:, ls], oT[p][:, ls],
                                         bc[:, 0:512])
                    yield

            def chain(*gens):
                for g in gens:
                    yield from g

            # ---- emission ----
            load_x("q", xqT)
            load_w("q", wqT)
            load_bias("q", bq)
            load_x("k", xkT)
            load_w("k", wkT)
            load_bias("k", bk)
            load_x("v", xvT)
            load_w("v", wvT)
            for j in range(EC):
                wt = persist.tile([128, E], F16, name=f"wo16_{j}")
                nc.sync.dma_start(wt[:], woT[j * 128:(j + 1) * 128, :])
                wo16.append(wt)
            # only pair 0's q/k projections run before the first scores;
            # the rest hide in group (0,0)'s ScalarE-bound stretch
            for _ in proj_qk("q", 0):
                pass
            for _ in proj_qk("k", 0):
                pass

            def proj_rest():
                for st in range(LT):
                    proj_v(st)
                    yield
                yield from proj_qk("q", 1)
                yield from proj_qk("k", 1)
                yield from proj_qk("q", 2)
                yield from proj_qk("k", 2)

            # `pending` threads any undrained background into the next
            # group, so generators never get dropped mid-stream
            pending = proj_rest()
            groups = [(p, lq) for p in range(EC) for lq in range(4)]
            for gi, (p, lq) in enumerate(groups):
                if gi == 0:
                    pass
                elif lq > 0:
                    pending = chain(pending, attnv(2 * p, lq - 1),
                                    attnv(2 * p + 1, lq - 1))
                else:
                    pending = chain(pending, attnv(2 * p - 2, 3),
                                    attnv(2 * p - 1, 3), norm(p - 1))
                scores_exp(p, lq, pending)
            for _ in chain(pending, attnv(4, 3), attnv(5, 3), norm(2)):
                pass

            # ---- out-projection ----
            for lc in range(4):
                ls = slice(lc * 512, (lc + 1) * 512)
                for eo in range(6):
                    po = psum.tile([128, 1024], F32, tag="av", bufs=2,
                                   name="mm_out")
                    for j in range(EC):
                        nc.tensor.matmul(
                            po[:, 0:512], wo16[j][:, eo * 128:(eo + 1) * 128],
                            oT[j][:, ls],
                            start=(j == 0), stop=(j == EC - 1))
                    osb = outsb_pool.tile([128, 512], F16, tag="osb", bufs=4,
                                          name="osb")
                    nc.vector.tensor_copy(osb[:], po[:, 0:512])
                    nc.sync.dma_start(
                        out[eo * 128:(eo + 1) * 128, ls], osb[:])
    nc.finalize()
    return nc


def kernel(query, key, value, in_proj_weight, in_proj_bias,
           q_down, q_up, k_down, k_up, v_down, v_up,
           out_proj_weight, out_proj_bias, out_down, out_up):
    if "nc" not in _CACHED:
        _CACHED["nc"] = _build()
    nc = _CACHED["nc"]

    f = np.float32
    h = np.float16
    w_eff = {}
    for i, (dn, up) in enumerate(((q_down, q_up), (k_down, k_up),
                                  (v_down, v_up))):
        w = in_proj_weight[i * E:(i + 1) * E].astype(f)
        w_eff[i] = w + LORA_SCALE * (up.astype(f) @ dn.astype(f))
    wo_eff = out_proj_weight.astype(f) + LORA_SCALE * (
        out_up.astype(f) @ out_down.astype(f))

    in_maps = []
    for c in range(NC_):
        n, hg = c // 2, c % 2
        sl = slice(hg * EG, (hg + 1) * EG)
        m = {
            "xqT": np.ascontiguousarray(query[:, n, :].T, dtype=h),
            "xkT": np.ascontiguousarray(key[:, n, :].T, dtype=h),
            "xvT": np.ascontiguousarray(value[:, n, :].T, dtype=h),
            "wqT": np.ascontiguousarray(w_eff[0][sl].T, dtype=h),
            "wkT": np.ascontiguousarray(w_eff[1][sl].T, dtype=h),
            "wvT": np.ascontiguousarray(w_eff[2][sl].T, dtype=h),
            "woT": np.ascontiguousarray(wo_eff[:, sl].T, dtype=h),
            "bq": np.ascontiguousarray(in_proj_bias[0:E][sl], dtype=f),
            "bk": np.ascontiguousarray(in_proj_bias[E:2 * E][sl], dtype=f),
        }
        in_maps.append(m)

    _CACHED["in_maps"] = in_maps
    res = run_bass_kernel_spmd(nc, in_maps, list(range(NC_)))
    outp = np.empty((L, N, E), dtype=np.float32)
    bo_total = out_proj_bias.astype(f) + wo_eff @ np.ascontiguousarray(
        in_proj_bias[2 * E:3 * E], dtype=f)
    for n in range(N):
        outp[:, n, :] = (res.results[2 * n]["out"].astype(f)
                         + res.results[2 * n + 1]["out"].astype(f)).T + bo_total
    return outp


# revision 36
# speedup vs baseline: 1.0299x; 1.0299x over previous
"""LoRA MultiheadAttention on 8 Trainium2 NeuronCores (Bass/Tile) — v3.

Sharding: core c = (batch n = c//2, head-group hg = c%2); each core handles
6 of 12 heads for one of 4 batches. LoRA is folded into the projection
weights on the host (exact identity). f16 activations/weights from host;
f16 output partials summed on host.

v3 structure (vs v2 @ 387µs): every TensorE stage is arranged so matmuls
alternate PE row groups, which (a) lets LDWEIGHTS of one matmul overlap
the previous matmul's streaming (they serialize when row groups collide:
~107ns tax per matmul in v2) and (b) lets pairs of K=64 matmuls stream
CONCURRENTLY through complementary halves of the PE array:
- Scores: the head pair (2p, 2p+1) lives at qkT partitions 0-63/64-127,
  so interleaving the two heads' score matmuls runs them on row groups
  0-1/2-3 concurrently. Processed in l-quarter groups of st-pairs so the
  attention tiles for one l-quarter can be consumed (attnV) while the
  next fills — bounds SBUF to ~2 groups.
- Projections / attnV / out-proj contract over K=128: split into K=64
  halves accumulated into SEPARATE psum half-tiles (cross-row-group
  accumulation into one psum region is a device error — probed), then
  combined by the DVE op that was already there (bias-add / output copy).
- Softmax denominators ride as a 65th v row; reciprocal via
  reciprocal_approx_fast on a pair-packed [64, L] tile; broadcast to the
  128 output rows via one K=64 matmul against a constant selection
  matrix; all partition starts 32-aligned.
- attnV chains and normalization are generator-interleaved into the
  next group's scores/exp emission so TensorE works through the
  ScalarE-bound exp stretches.
"""
import numpy as np

import concourse.bass as bass
import concourse.tile as tile
from concourse import bacc, mybir
from concourse.bass_utils import run_bass_kernel_spmd

L, N, E, H, R = 2048, 4, 768, 12, 16
ALPHA = 16.0
LORA_SCALE = ALPHA / R
HD = E // H          # 64
HG = 2
HPG = H // HG        # 6 heads per group
EG = E // HG         # 384
NC_ = 8
F32 = mybir.dt.float32
F16 = mybir.dt.float16
SCALE = 1.0 / float(np.sqrt(HD))
ADD = mybir.AluOpType.add

KC = E // 128        # 6 contraction chunks
EC = EG // 128       # 3 output chunks (= head pairs)
LT = L // 128        # 16 s tiles
VW = HPG * (HD + 1)  # 390

_CACHED = {}


def _build():
    nc = bacc.Bacc()
    xqT = nc.dram_tensor("xqT", [E, L], F16, kind="ExternalInput")
    xkT = nc.dram_tensor("xkT", [E, L], F16, kind="ExternalInput")
    xvT = nc.dram_tensor("xvT", [E, L], F16, kind="ExternalInput")
    wqT = nc.dram_tensor("wqT", [E, EG], F16, kind="ExternalInput")
    wkT = nc.dram_tensor("wkT", [E, EG], F16, kind="ExternalInput")
    wvT = nc.dram_tensor("wvT", [E, EG], F16, kind="ExternalInput")
    woT = nc.dram_tensor("woT", [EG, E], F16, kind="ExternalInput")
    bq = nc.dram_tensor("bq", [EG], F32, kind="ExternalInput")
    bk = nc.dram_tensor("bk", [EG], F32, kind="ExternalInput")
    out = nc.dram_tensor("out", [E, L], F16, kind="ExternalOutput")

    with tile.TileContext(nc) as tc:
        with (
            tc.tile_pool(name="big", bufs=44) as big,
            tc.tile_pool(name="persist", bufs=1) as persist,
            tc.tile_pool(name="small", bufs=1) as small,
            tc.tile_pool(name="outsb", bufs=4) as outsb_pool,
            tc.tile_pool(name="psum", bufs=1, space="PSUM") as psum,
        ):
            LO, HI = slice(0, 64), slice(64, 128)

            # ---- weights / constants (x DMAs are issued first, in
            # load_x below, so the first projection isn't queued behind
            # weight traffic it doesn't need yet) ----
            w16 = {}
            def load_w(pname, wdram):
                for j in range(KC):
                    wt = persist.tile([128, EG], F16, name=f"w16_{pname}{j}")
                    nc.sync.dma_start(wt[:], wdram[j * 128:(j + 1) * 128, :])
                    w16[pname, j] = wt
            wo16 = []
            bias_t = {}
            def load_bias(bname, bdram):
                for j in range(EC):
                    bt = persist.tile([128, 1], F32, name=f"b_{bname}{j}")
                    nc.sync.dma_start(bt[:], bdram[j * 128:(j + 1) * 128])
                    bias_t[bname, j] = bt
            esel = persist.tile([64, 128], F16, name="esel")
            nc.vector.memset(esel[:], 0.0)
            nc.vector.memset(esel[0:1, 0:64], 1.0)
            nc.vector.memset(esel[32:33, 64:128], 1.0)

            qkT = {}
            v_aug = [None] * LT
            oT = [persist.tile([128, L], F16, name=f"oT{j}")
                  for j in range(EC)]
            d2 = {p: small.tile([64, L], F32, name=f"d2_{p}")
                  for p in range(EC)}
            for p in range(EC):
                nc.vector.memset(d2[p][:], 1.0)
            attn = {}    # (head, stp, lq) -> [128, 1024] f16: st-pair halves

            # ---- input staging: [128, 1024] ring tiles, 2 per E-chunk ----
            x16 = {}
            def load_x(pname, xdram):
                for j in range(KC):
                    for lh in range(2):
                        xt = big.tile([128, 1024], F16, tag="big", name="x16")
                        nc.sync.dma_start(
                            xt[:], xdram[j * 128:(j + 1) * 128,
                                         lh * 1024:(lh + 1) * 1024])
                        x16[pname, j, lh] = xt

            def proj_qk(pname, e):
                dst = persist.tile([128, L], F16, name=f"{pname}T{e}")
                qkT[pname, e] = dst
                for lc in range(2):
                    mm = psum.tile([128, 1024], F32, tag="sc", bufs=2,
                                   name="mm_proj")
                    for half in range(2):
                        xsl = slice(half * 512, (half + 1) * 512)
                        for kk in range(KC):
                            nc.tensor.matmul(
                                mm[:, xsl],
                                w16[pname, kk][:, e * 128:(e + 1) * 128],
                                x16[pname, kk, lc][:, xsl],
                                start=(kk == 0), stop=(kk == KC - 1))
                    nc.vector.tensor_scalar_add(
                        dst[:, lc * 1024:(lc + 1) * 1024], mm[:],
                        bias_t[pname, e][:])

            def proj_v(st):
                mm = psum.tile([128, 1024], F32, tag="sc", bufs=2,
                               name="mm_vproj")
                lh, xo = st // 8, (st % 8) * 128
                for kk in range(KC):
                    nc.tensor.matmul(
                        mm[:, 0:EG], x16["v", kk, lh][:, xo:xo + 128],
                        w16["v", kk][:],
                        start=(kk == 0), stop=(kk == KC - 1))
                vt = persist.tile([128, VW], F16, name=f"v_aug{st}")
                grp = vt.rearrange("p (h c) -> p h c", c=HD + 1)
                nc.vector.tensor_copy(
                    grp[:, :, 0:HD],
                    mm[:, 0:EG].rearrange("p (h c) -> p h c", c=HD))
                nc.vector.memset(grp[:, :, HD:HD + 1], 1.0)
                v_aug[st] = vt

            def scores_exp(p, lq, background):
                """Scores+exp for head pair p over l-quarter lq.

                Both heads' score matmuls write halves of ONE psum tile so
                they issue back-to-back with no inter-tile semaphore — the
                row-group-complementary pair (qkT partitions 0-63 / 64-127)
                then streams concurrently through the PE array.
                """
                qk = [(qkT["q", p][j * 64:(j + 1) * 64, :],
                       qkT["k", p][j * 64:(j + 1) * 64, :]) for j in range(2)]
                ls = slice(lq * 512, (lq + 1) * 512)
                for st in range(LT):
                    T = psum.tile([128, 1024], F32, tag="sc", bufs=2,
                                  name="mm_sc")
                    for j in range(2):
                        qs, ks = qk[j]
                        nc.tensor.matmul(
                            T[:, j * 512:(j + 1) * 512],
                            ks[:, st * 128:(st + 1) * 128],
                            qs[:, ls],
                            start=True, stop=True)
                    at = big.tile([128, 1024], F16, tag="big", name="attn")
                    attn[p, st, lq] = at
                    nc.scalar.activation(
                        at[:], T[:],
                        mybir.ActivationFunctionType.Exp, scale=SCALE)
                    for _ in range(3):
                        next(background, None)

            def attnv(h, lq):
                """attnV chains for head h, l-quarter lq (split-K halves)."""
                et, j = h // 2, h % 2
                vsl = slice(h * (HD + 1), (h + 1) * (HD + 1))
                ch = psum.tile([128, 1024], F32, tag="av", bufs=2, name="ch")
                for st in range(LT):
                    at = attn[et, st, lq]
                    nc.tensor.matmul(
                        ch[0:65, 0:512], v_aug[st][:, vsl],
                        at[:, j * 512:(j + 1) * 512],
                        start=(st == 0), stop=(st == LT - 1))
                    yield
                ls = slice(lq * 512, (lq + 1) * 512)
                nc.vector.tensor_copy(
                    oT[et][j * 64:(j + 1) * 64, ls], ch[0:64, 0:512])
                nc.vector.tensor_copy(
                    d2[et][32 * j:32 * j + 1, ls], ch[64:65, 0:512])
                yield

            def norm_lq(p, lq):
                """Normalize pair p's oT rows for one l-quarter as soon as
                both heads' denominator chunks land — lets the out-proj
                for that l-quarter pipeline into pair 2's score groups."""
                ls = slice(lq * 512, (lq + 1) * 512)
                rec32 = small.tile([64, 512], F32, tag="rec32", bufs=2,
                                   name="rec32")
                rec2 = small.tile([64, 512], F16, tag="rec2", bufs=2,
                                  name="rec2")
                nc.vector.reciprocal_approx_fast(rec32[:], d2[p][:, ls])
                nc.vector.tensor_copy(rec2[:], rec32[:])
                yield
                bc = psum.tile([128, 1024], F32, tag="av", bufs=2,
                               name="bc")
                nc.tensor.matmul(bc[:, 0:512], esel[:], rec2[:],
                                 start=True, stop=True)
                nc.vector.tensor_mul(oT[p][:, ls], oT[p][:, ls],
                                     bc[:, 0:512])
                yield

            def outproj_lc(lc):
                ls = slice(lc * 512, (lc + 1) * 512)
                for eo in range(6):
                    po = psum.tile([128, 1024], F32, tag="av", bufs=2,
                                   name="mm_out")
                    for j in range(EC):
                        nc.tensor.matmul(
                            po[:, 0:512], wo16[j][:, eo * 128:(eo + 1) * 128],
                            oT[j][:, ls],
                            start=(j == 0), stop=(j == EC - 1))
                    osb = outsb_pool.tile([128, 512], F16, tag="osb", bufs=4,
                                          name="osb")
                    nc.vector.tensor_copy(osb[:], po[:, 0:512])
                    nc.sync.dma_start(
                        out[eo * 128:(eo + 1) * 128, ls], osb[:])
                    yield

            def chain(*gens):
                for g in gens:
                    yield from g

            # ---- emission ----
            load_x("q", xqT)
            load_w("q", wqT)
            load_bias("q", bq)
            load_x("k", xkT)
            load_w("k", wkT)
            load_bias("k", bk)
            for e in range(EC):
                proj_qk("q", e)
                proj_qk("k", e)
            load_x("v", xvT)
            load_w("v", wvT)
            for j in range(EC):
                wt = persist.tile([128, E], F16, name=f"wo16_{j}")
                nc.sync.dma_start(wt[:], woT[j * 128:(j + 1) * 128, :])
                wo16.append(wt)
            for st in range(LT):
                proj_v(st)

            empty = iter(())
            groups = [(p, lq) for p in range(EC) for lq in range(4)]
            for gi, (p, lq) in enumerate(groups):
                if gi == 0:
                    bg = empty
                elif lq > 0:
                    bg = chain(attnv(2 * p, lq - 1), attnv(2 * p + 1, lq - 1),
                               norm_lq(p, lq - 1),
                               *([outproj_lc(lq - 1)] if p == 2 else []))
                else:
                    bg = chain(attnv(2 * p - 2, 3), attnv(2 * p - 1, 3),
                               norm_lq(p - 1, 3))
                scores_exp(p, lq, bg)
            for _ in chain(attnv(4, 3), attnv(5, 3), norm_lq(2, 3),
                           outproj_lc(3)):
                pass
    nc.finalize()
    return nc


def kernel(query, key, value, in_proj_weight, in_proj_bias,
           q_down, q_up, k_down, k_up, v_down, v_up,
           out_proj_weight, out_proj_bias, out_down, out_up):
    if "nc" not in _CACHED:
        _CACHED["nc"] = _build()
    nc = _CACHED["nc"]

    f = np.float32
    h = np.float16
    w_eff = {}
    for i, (dn, up) in enumerate(((q_down, q_up), (k_down, k_up),
                                  (v_down, v_up))):
        w = in_proj_weight[i * E:(i + 1) * E].astype(f)
        w_eff[i] = w + LORA_SCALE * (up.astype(f) @ dn.astype(f))
    wo_eff = out_proj_weight.astype(f) + LORA_SCALE * (
        out_up.astype(f) @ out_down.astype(f))

    in_maps = []
    for c in range(NC_):
        n, hg = c // 2, c % 2
        sl = slice(hg * EG, (hg + 1) * EG)
        m = {
            "xqT": np.ascontiguousarray(query[:, n, :].T, dtype=h),
            "xkT": np.ascontiguousarray(key[:, n, :].T, dtype=h),
            "xvT": np.ascontiguousarray(value[:, n, :].T, dtype=h),
            "wqT": np.ascontiguousarray(w_eff[0][sl].T, dtype=h),
            "wkT": np.ascontiguousarray(w_eff[1][sl].T, dtype=h),
            "wvT": np.ascontiguousarray(w_eff[2][sl].T, dtype=h),
            "woT": np.ascontiguousarray(wo_eff[:, sl].T, dtype=h),
            "bq": np.ascontiguousarray(in_proj_bias[0:E][sl], dtype=f),
            "bk": np.ascontiguousarray(in_proj_bias[E:2 * E][sl], dtype=f),
        }
        in_maps.append(m)

    _CACHED["in_maps"] = in_maps
    res = run_bass_kernel_spmd(nc, in_maps, list(range(NC_)))
    outp = np.empty((L, N, E), dtype=np.float32)
    bo_total = out_proj_bias.astype(f) + wo_eff @ np.ascontiguousarray(
        in_proj_bias[2 * E:3 * E], dtype=f)
    for n in range(N):
        outp[:, n, :] = (res.results[2 * n]["out"].astype(f)
                         + res.results[2 * n + 1]["out"].astype(f)).T + bo_total
    return outp


# revision 37
# speedup vs baseline: 1.0361x; 1.0060x over previous
"""LoRA MultiheadAttention on 8 Trainium2 NeuronCores (Bass/Tile) — v3.

Sharding: core c = (batch n = c//2, head-group hg = c%2); each core handles
6 of 12 heads for one of 4 batches. LoRA is folded into the projection
weights on the host (exact identity). f16 activations/weights from host;
f16 output partials summed on host.

v3 structure (vs v2 @ 387µs): every TensorE stage is arranged so matmuls
alternate PE row groups, which (a) lets LDWEIGHTS of one matmul overlap
the previous matmul's streaming (they serialize when row groups collide:
~107ns tax per matmul in v2) and (b) lets pairs of K=64 matmuls stream
CONCURRENTLY through complementary halves of the PE array:
- Scores: the head pair (2p, 2p+1) lives at qkT partitions 0-63/64-127,
  so interleaving the two heads' score matmuls runs them on row groups
  0-1/2-3 concurrently. Processed in l-quarter groups of st-pairs so the
  attention tiles for one l-quarter can be consumed (attnV) while the
  next fills — bounds SBUF to ~2 groups.
- Projections / attnV / out-proj contract over K=128: split into K=64
  halves accumulated into SEPARATE psum half-tiles (cross-row-group
  accumulation into one psum region is a device error — probed), then
  combined by the DVE op that was already there (bias-add / output copy).
- Softmax denominators ride as a 65th v row; reciprocal via
  reciprocal_approx_fast on a pair-packed [64, L] tile; broadcast to the
  128 output rows via one K=64 matmul against a constant selection
  matrix; all partition starts 32-aligned.
- attnV chains and normalization are generator-interleaved into the
  next group's scores/exp emission so TensorE works through the
  ScalarE-bound exp stretches.
"""
import numpy as np

import concourse.bass as bass
import concourse.tile as tile
from concourse import bacc, mybir
from concourse.bass_utils import run_bass_kernel_spmd

L, N, E, H, R = 2048, 4, 768, 12, 16
ALPHA = 16.0
LORA_SCALE = ALPHA / R
HD = E // H          # 64
HG = 2
HPG = H // HG        # 6 heads per group
EG = E // HG         # 384
NC_ = 8
F32 = mybir.dt.float32
F16 = mybir.dt.float16
SCALE = 1.0 / float(np.sqrt(HD))
ADD = mybir.AluOpType.add

KC = E // 128        # 6 contraction chunks
EC = EG // 128       # 3 output chunks (= head pairs)
LT = L // 128        # 16 s tiles
VW = HPG * (HD + 1)  # 390

_CACHED = {}


def _build():
    nc = bacc.Bacc()
    xqT = nc.dram_tensor("xqT", [E, L], F16, kind="ExternalInput")
    xkT = nc.dram_tensor("xkT", [E, L], F16, kind="ExternalInput")
    xvT = nc.dram_tensor("xvT", [E, L], F16, kind="ExternalInput")
    wqT = nc.dram_tensor("wqT", [E, EG], F16, kind="ExternalInput")
    wkT = nc.dram_tensor("wkT", [E, EG], F16, kind="ExternalInput")
    wvT = nc.dram_tensor("wvT", [E, EG], F16, kind="ExternalInput")
    woT = nc.dram_tensor("woT", [EG, E], F16, kind="ExternalInput")
    bq = nc.dram_tensor("bq", [EG], F32, kind="ExternalInput")
    bk = nc.dram_tensor("bk", [EG], F32, kind="ExternalInput")
    out = nc.dram_tensor("out", [E, L], F16, kind="ExternalOutput")

    with tile.TileContext(nc) as tc:
        with (
            tc.tile_pool(name="big", bufs=44) as big,
            tc.tile_pool(name="persist", bufs=1) as persist,
            tc.tile_pool(name="small", bufs=1) as small,
            tc.tile_pool(name="outsb", bufs=4) as outsb_pool,
            tc.tile_pool(name="psum", bufs=1, space="PSUM") as psum,
        ):
            LO, HI = slice(0, 64), slice(64, 128)

            # ---- weights / constants (x DMAs are issued first, in
            # load_x below, so the first projection isn't queued behind
            # weight traffic it doesn't need yet) ----
            w16 = {}
            def load_w(pname, wdram):
                for j in range(KC):
                    wt = persist.tile([128, EG], F16, name=f"w16_{pname}{j}")
                    nc.sync.dma_start(wt[:], wdram[j * 128:(j + 1) * 128, :])
                    w16[pname, j] = wt
            wo16 = []
            bias_t = {}
            def load_bias(bname, bdram):
                for j in range(EC):
                    bt = persist.tile([128, 1], F32, name=f"b_{bname}{j}")
                    nc.sync.dma_start(bt[:], bdram[j * 128:(j + 1) * 128])
                    bias_t[bname, j] = bt
            esel = persist.tile([64, 128], F16, name="esel")
            nc.vector.memset(esel[:], 0.0)
            nc.vector.memset(esel[0:1, 0:64], 1.0)
            nc.vector.memset(esel[32:33, 64:128], 1.0)

            qkT = {}
            v_aug = [None] * LT
            oT = [persist.tile([128, L], F16, name=f"oT{j}")
                  for j in range(EC)]
            d2 = {p: small.tile([64, L], F32, name=f"d2_{p}")
                  for p in range(EC)}
            for p in range(EC):
                nc.vector.memset(d2[p][:], 1.0)
            attn = {}    # (head, stp, lq) -> [128, 1024] f16: st-pair halves

            # ---- input staging: [128, 1024] ring tiles, 2 per E-chunk ----
            x16 = {}
            def load_x(pname, xdram):
                for j in range(KC):
                    for lh in range(2):
                        xt = big.tile([128, 1024], F16, tag="big", name="x16")
                        nc.sync.dma_start(
                            xt[:], xdram[j * 128:(j + 1) * 128,
                                         lh * 1024:(lh + 1) * 1024])
                        x16[pname, j, lh] = xt

            def proj_qk(pname, e):
                dst = persist.tile([128, L], F16, name=f"{pname}T{e}")
                qkT[pname, e] = dst
                for lc in range(2):
                    mm = psum.tile([128, 1024], F32, tag="sc", bufs=2,
                                   name="mm_proj")
                    for half in range(2):
                        xsl = slice(half * 512, (half + 1) * 512)
                        for kk in range(KC):
                            nc.tensor.matmul(
                                mm[:, xsl],
                                w16[pname, kk][:, e * 128:(e + 1) * 128],
                                x16[pname, kk, lc][:, xsl],
                                start=(kk == 0), stop=(kk == KC - 1))
                    nc.vector.tensor_scalar_add(
                        dst[:, lc * 1024:(lc + 1) * 1024], mm[:],
                        bias_t[pname, e][:])

            def proj_v(st):
                mm = psum.tile([128, 1024], F32, tag="sc", bufs=2,
                               name="mm_vproj")
                lh, xo = st // 8, (st % 8) * 128
                for kk in range(KC):
                    nc.tensor.matmul(
                        mm[:, 0:EG], x16["v", kk, lh][:, xo:xo + 128],
                        w16["v", kk][:],
                        start=(kk == 0), stop=(kk == KC - 1))
                vt = persist.tile([128, VW], F16, name=f"v_aug{st}")
                grp = vt.rearrange("p (h c) -> p h c", c=HD + 1)
                nc.vector.tensor_copy(
                    grp[:, :, 0:HD],
                    mm[:, 0:EG].rearrange("p (h c) -> p h c", c=HD))
                nc.vector.memset(grp[:, :, HD:HD + 1], 1.0)
                v_aug[st] = vt

            def scores_exp(p, lq, background):
                """Scores+exp for head pair p over l-quarter lq.

                Both heads' score matmuls write halves of ONE psum tile so
                they issue back-to-back with no inter-tile semaphore — the
                row-group-complementary pair (qkT partitions 0-63 / 64-127)
                then streams concurrently through the PE array.
                """
                qk = [(qkT["q", p][j * 64:(j + 1) * 64, :],
                       qkT["k", p][j * 64:(j + 1) * 64, :]) for j in range(2)]
                ls = slice(lq * 512, (lq + 1) * 512)
                for st in range(LT):
                    T = psum.tile([128, 1024], F32, tag="sc", bufs=2,
                                  name="mm_sc")
                    for j in range(2):
                        qs, ks = qk[j]
                        nc.tensor.matmul(
                            T[:, j * 512:(j + 1) * 512],
                            ks[:, st * 128:(st + 1) * 128],
                            qs[:, ls],
                            start=True, stop=True)
                    at = big.tile([128, 1024], F16, tag="big", name="attn")
                    attn[p, st, lq] = at
                    nc.scalar.activation(
                        at[:], T[:],
                        mybir.ActivationFunctionType.Exp, scale=SCALE)
                    for _ in range(3):
                        next(background, None)

            def attnv(h, lq):
                """attnV chains for head h, l-quarter lq (split-K halves)."""
                et, j = h // 2, h % 2
                vsl = slice(h * (HD + 1), (h + 1) * (HD + 1))
                ch = psum.tile([128, 1024], F32, tag="av", bufs=2, name="ch")
                for st in range(LT):
                    at = attn[et, st, lq]
                    nc.tensor.matmul(
                        ch[0:65, 0:512], v_aug[st][:, vsl],
                        at[:, j * 512:(j + 1) * 512],
                        start=(st == 0), stop=(st == LT - 1))
                    yield
                ls = slice(lq * 512, (lq + 1) * 512)
                nc.vector.tensor_copy(
                    oT[et][j * 64:(j + 1) * 64, ls], ch[0:64, 0:512])
                nc.vector.tensor_copy(
                    d2[et][32 * j:32 * j + 1, ls], ch[64:65, 0:512])
                yield

            def norm(p):
                rec32 = small.tile([64, L], F32, tag="rec32", bufs=1,
                                   name="rec32")
                rec2 = small.tile([64, L], F16, tag="rec2", bufs=1,
                                  name="rec2")
                nc.vector.reciprocal_approx_fast(rec32[:], d2[p][:])
                nc.vector.tensor_copy(rec2[:], rec32[:])
                yield
                for lq in range(4):
                    bc = psum.tile([128, 1024], F32, tag="av", bufs=2,
                                   name="bc")
                    ls = slice(lq * 512, (lq + 1) * 512)
                    nc.tensor.matmul(bc[:, 0:512], esel[:], rec2[:, ls],
                                     start=True, stop=True)
                    nc.vector.tensor_mul(oT[p][:, ls], oT[p][:, ls],
                                         bc[:, 0:512])
                    yield

            def chain(*gens):
                for g in gens:
                    yield from g

            # ---- emission ----
            load_x("q", xqT)
            load_w("q", wqT)
            load_bias("q", bq)
            load_x("k", xkT)
            load_w("k", wkT)
            load_bias("k", bk)
            for e in range(EC):
                proj_qk("q", e)
                proj_qk("k", e)
            load_x("v", xvT)
            load_w("v", wvT)
            for j in range(EC):
                wt = persist.tile([128, E], F16, name=f"wo16_{j}")
                nc.sync.dma_start(wt[:], woT[j * 128:(j + 1) * 128, :])
                wo16.append(wt)
            for st in range(LT):
                proj_v(st)

            empty = iter(())
            groups = [(p, lq) for p in range(EC) for lq in range(4)]
            for gi, (p, lq) in enumerate(groups):
                if gi == 0:
                    bg = empty
                elif lq > 0:
                    bg = chain(attnv(2 * p, lq - 1), attnv(2 * p + 1, lq - 1))
                else:
                    bg = chain(attnv(2 * p - 2, 3), attnv(2 * p - 1, 3),
                               norm(p - 1))
                scores_exp(p, lq, bg)
            for _ in chain(attnv(4, 3), attnv(5, 3), norm(2)):
                pass

            # ---- out-projection ----
            for lc in range(4):
                ls = slice(lc * 512, (lc + 1) * 512)
                for eo in range(6):
                    po = psum.tile([128, 1024], F32, tag="av", bufs=2,
                                   name="mm_out")
                    for j in range(EC):
                        nc.tensor.matmul(
                            po[:, 0:512], wo16[j][:, eo * 128:(eo + 1) * 128],
                            oT[j][:, ls],
                            start=(j == 0), stop=(j == EC - 1))
                    osb = outsb_pool.tile([128, 512], F16, tag="osb", bufs=4,
                                          name="osb")
                    nc.vector.tensor_copy(osb[:], po[:, 0:512])
                    nc.sync.dma_start(
                        out[eo * 128:(eo + 1) * 128, ls], osb[:])
    nc.finalize()
    return nc


def kernel(query, key, value, in_proj_weight, in_proj_bias,
           q_down, q_up, k_down, k_up, v_down, v_up,
           out_proj_weight, out_proj_bias, out_down, out_up):
    if "nc" not in _CACHED:
        _CACHED["nc"] = _build()
    nc = _CACHED["nc"]

    f = np.float32
    h = np.float16
    w_eff = {}
    for i, (dn, up) in enumerate(((q_down, q_up), (k_down, k_up),
                                  (v_down, v_up))):
        w = in_proj_weight[i * E:(i + 1) * E].astype(f)
        w_eff[i] = w + LORA_SCALE * (up.astype(f) @ dn.astype(f))
    wo_eff = out_proj_weight.astype(f) + LORA_SCALE * (
        out_up.astype(f) @ out_down.astype(f))

    in_maps = []
    for c in range(NC_):
        n, hg = c // 2, c % 2
        sl = slice(hg * EG, (hg + 1) * EG)
        m = {
            "xqT": np.ascontiguousarray(query[:, n, :].T, dtype=h),
            "xkT": np.ascontiguousarray(key[:, n, :].T, dtype=h),
            "xvT": np.ascontiguousarray(value[:, n, :].T, dtype=h),
            "wqT": np.ascontiguousarray(w_eff[0][sl].T, dtype=h),
            "wkT": np.ascontiguousarray(w_eff[1][sl].T, dtype=h),
            "wvT": np.ascontiguousarray(w_eff[2][sl].T, dtype=h),
            "woT": np.ascontiguousarray(wo_eff[:, sl].T, dtype=h),
            "bq": np.ascontiguousarray(in_proj_bias[0:E][sl], dtype=f),
            "bk": np.ascontiguousarray(in_proj_bias[E:2 * E][sl], dtype=f),
        }
        in_maps.append(m)

    _CACHED["in_maps"] = in_maps
    res = run_bass_kernel_spmd(nc, in_maps, list(range(NC_)))
    outp = np.empty((L, N, E), dtype=np.float32)
    bo_total = out_proj_bias.astype(f) + wo_eff @ np.ascontiguousarray(
        in_proj_bias[2 * E:3 * E], dtype=f)
    for n in range(N):
        outp[:, n, :] = (res.results[2 * n]["out"].astype(f)
                         + res.results[2 * n + 1]["out"].astype(f)).T + bo_total
    return outp
